# revision 15
# baseline (speedup 1.0000x reference)
"""Trainium2 Bass kernel for CdfgReader GNN message passing.

Strategy:
  - 64 batch items draw from <=32 unique CDFGs: compute the GNN once per
    unique graph; distribute ceil(u/8) graph slots per core across 8 cores
    (SPMD, one compiled program specialized to the input's structure).
  - Error budget (tolerance 2e-2): the end-to-end error is dominated by the
    f32r rounding of the *weights* (a systematic perturbation); activation
    rounding averages out through the A-multiply and the masked mean.
    So W_in/W_gcn ship as f32r hi+lo pairs (every X@W does 2 matmuls per
    contraction tile), while activations stay single f32r and every A-mult
    runs once.  Measured end-to-end ~1.3e-3.
  - Per slot: X0^T = relu(W^T xs^T) h-major; 3x { XW node-major pipelined
    m-outer with the h-major A-multiply X^T = relu(XW^T A^T) }; the final
    layer is computed node-major only for the first K_g 128-node tiles,
    where the host permutes each graph's nodes so the union of its
    coverpoint masks comes first.  The residual relu(xs@W_in+b) is
    recomputed node-major for those K_g tiles directly from xs^T (cheap,
    and it fills the PE while the first slot's A matrix streams in), and
    the masked sums use a small mask matmul.  DMAs are ordered so the
    input-layer operands land first; A streams per 128-row chunk.
"""

import numpy as np

NCORES = 8
N = 1024        # max nodes
F = 128         # input feature dim
H = 256         # hidden dim
L = 4           # GCN layers
B = 64          # batch (coverpoints)

_CACHE = {}


def _rnd11(x):
    # round-to-nearest-even at 11 explicit mantissa bits (f32r-exact)
    m, e = np.frexp(np.float32(x))
    m = np.round(m * 4096.0) / 4096.0
    return np.ldexp(m, e).astype(np.float32)


def _build_nc(NG, Ks):
    import concourse.bass as bass  # noqa: F401
    import concourse.mybir as mybir
    import concourse.tile as tile
    from concourse import bacc
    from concourse.bass import ts

    f32 = mybir.dt.float32
    f32r = mybir.dt.float32r
    Relu = mybir.ActivationFunctionType.Relu
    Tanh = mybir.ActivationFunctionType.Tanh
    add = mybir.AluOpType.add

    T = sum(Ks)
    offs = np.concatenate([[0], np.cumsum(Ks)]).astype(int)
    Kmax = max(Ks)

    nc = bacc.Bacc("TRN2", target_bir_lowering=False, debug=False,
                   num_devices=NCORES)

    a_t = nc.dram_tensor("a_t", [NG, N, N], f32r, kind="ExternalInput")
    xs_t = nc.dram_tensor("xs_t", [F, NG, N], f32r, kind="ExternalInput")
    m_t = nc.dram_tensor("m_t", [128, T, B], f32r, kind="ExternalInput")
    w_in_hi = nc.dram_tensor("w_in_hi", [F, H], f32r, kind="ExternalInput")
    # W_gcn packed [128, (l,t), hi/lo, H] so layer slices are single DMAs
    w_gcn = nc.dram_tensor("w_gcn", [128, L * 2, 2, H], f32r,
                           kind="ExternalInput")
    b_pp = nc.dram_tensor("b_pp", [128, 2 + L * 2], f32, kind="ExternalInput")
    rows_c = nc.dram_tensor("rows_c", [1, 2 * H + 128], f32r,
                            kind="ExternalInput")
    out = nc.dram_tensor("out", [B, H], f32, kind="ExternalOutput")

    with tile.TileContext(nc) as tc:
        with (
            tc.tile_pool(name="const", bufs=1) as constp,
            tc.tile_pool(name="apool", bufs=2) as apool,
            tc.tile_pool(name="xspool", bufs=2) as xspool,
            tc.tile_pool(name="x0pool", bufs=2) as x0pool,
            tc.tile_pool(name="x0npool", bufs=2) as x0npool,
            tc.tile_pool(name="xpool", bufs=2) as xpool,
            tc.tile_pool(name="xwpool", bufs=2) as xwpool,
            tc.tile_pool(name="xfpool", bufs=2) as xfpool,
            tc.tile_pool(name="psx", bufs=4, space="PSUM") as psx,
            tc.tile_pool(name="psw", bufs=3, space="PSUM") as psw,
        ):
            # --- DMA priority order: input-layer operands first, then the
            # first slot's A (chunked), then the rest.
            xs0 = xspool.tile([128, N], f32r, tag="xs", name="xs_g")
            nc.sync.dma_start(xs0[:, 0:512], xs_t[:, 0, 0:512])
            wi_hi_sb = constp.tile([128, H], f32r)
            nc.sync.dma_start(wi_hi_sb[:], w_in_hi[:, :])
            nc.sync.dma_start(xs0[:, 512:1024], xs_t[:, 0, 512:1024])
            b_pp_sb = constp.tile([128, 2 + L * 2], f32)
            nc.sync.dma_start(b_pp_sb[:], b_pp[:, :])
            rows_sb = constp.tile([1, 2 * H + 128], f32r)
            nc.sync.dma_start(rows_sb[:], rows_c[:, :])
            b_in_pp_sb = b_pp_sb[:, 0:2]
            b_gcn_pp_sb = b_pp_sb[:, 2:]
            b_in_row_sb = rows_sb[:, 0:H]
            b_g3_row_sb = rows_sb[:, H:2 * H]
            ones_sb = rows_sb[:, 2 * H:]

            w_sb = constp.tile([128, L * 2, 2, H], f32r)
            # layer-0 slices first (XW0 needs them before a_t finishes)
            nc.sync.dma_start(w_sb[:, 0:2, :, :], w_gcn[:, 0:2, :, :])

            a_sb0 = apool.tile([128, 8, N], f32r, tag="a", name="a_sb")
            for m in range(8):
                nc.sync.dma_start(a_sb0[:, m, :], a_t[0, ts(m, 128), :])

            nc.sync.dma_start(w_sb[:, 2:8, :, :], w_gcn[:, 2:8, :, :])
            m_t_sb = constp.tile([128, T, B], f32r)
            nc.sync.dma_start(m_t_sb[:], m_t[:, :, :])

            out_acc = constp.tile([B, H], f32)

            for g in range(NG):
                K = Ks[g]
                off = int(offs[g])
                if g == 0:
                    a_sb, xs_g = a_sb0, xs0
                else:
                    xs_g = xspool.tile([128, N], f32r, tag="xs", name="xs_g")
                    nc.sync.dma_start(xs_g[:], xs_t[:, g, :])
                    a_sb = apool.tile([128, 8, N], f32r, tag="a", name="a_sb")
                    for m in range(8):
                        nc.sync.dma_start(a_sb[:, m, :], a_t[g, ts(m, 128), :])

                # X0^T h-major [256h x 1024n], relu + bias on ACT
                x0t = x0pool.tile([128, 2, N], f32r, tag="x0")
                for t, c in [(0, 0), (1, 0), (0, 1), (1, 1)]:
                    ps = psx.tile([128, 512], f32, tag="psx", name="ps0")
                    nc.tensor.matmul(ps[:], wi_hi_sb[:, ts(t, 128)],
                                     xs_g[:, ts(c, 512)],
                                     start=True, stop=True)
                    nc.scalar.activation(x0t[:, t, ts(c, 512)], ps[:],
                                         Relu, bias=b_in_pp_sb[:, t:t + 1])

                # residual X0 node-major for the K masked tiles, straight
                # from xs^T; emitted in pieces as PE filler (all upfront for
                # slot 0 -- it hides under the initial A DMA -- else spread
                # across layer boundaries to cover the ACT handoff)
                x0n = x0npool.tile([128, Kmax, H], f32r, tag="x0n", name="x0n")

                def x0n_group(c):
                    ps = psw.tile([128, H], f32, tag="ps3", name="ps0n",
                                  bufs=2)
                    nc.tensor.matmul(ps[:], xs_g[:, ts(c, 128)], wi_hi_sb[:],
                                     start=True, stop=False)
                    nc.tensor.matmul(ps[:], ones_sb[:], b_in_row_sb[:],
                                     start=False, stop=True)
                    nc.scalar.activation(x0n[:, c, :], ps[:], Relu)

                cs = list(range(K))
                if g == 0:
                    # slot 0 is DMA-bound through layer 0: bulk up front,
                    # keep one group for each later layer boundary
                    x0n_layer = {0: [], 1: cs[K - 2:K - 1], 2: cs[K - 1:]}
                    head = cs[:K - 2]
                else:
                    nl = min(3, K - 1)
                    x0n_layer = {l: (cs[K - nl + l:K - nl + l + 1]
                                     if l < nl else [])
                                 for l in range(3)}
                    head = cs[:K - nl]
                for c in head:
                    x0n_group(c)

                x = x0t
                for layer in range(L - 1):
                    # XW node-major (W as hi+lo f32r pair).  The h-major
                    # A-multiply runs as two half-passes: pass A (c0 chunk)
                    # pipelines m-outer with the XW groups, pass B (c1)
                    # streams afterwards while the c0 ACTs drain, so the
                    # next layer's XW never waits on an ACT.
                    xw = xwpool.tile([128, 8, H], f32r, tag="xw", name="xw")
                    xn = xpool.tile([128, 2, N], f32r, tag="xn", name="xn")

                    def xw_pair(p):
                        # two m-tiles share one PSUM bank: one start/stop
                        # group, one wide copy -- halves ring turnarounds
                        ps = psw.tile([128, 2, H], f32, tag="psw",
                                      name="psw", bufs=2)
                        k = 0
                        for i in range(2):
                            for t in range(2):
                                for hl in range(2):
                                    nc.tensor.matmul(
                                        ps[:, i, :],
                                        x[:, t, ts(2 * p + i, 128)],
                                        w_sb[:, layer * 2 + t, hl, :],
                                        start=(k == 0), stop=(k == 7))
                                    k += 1
                        nc.vector.tensor_copy(xw[:, 2 * p:2 * p + 2, :],
                                              ps[:])

                    pssA = [psx.tile([128, 512], f32, tag="psx",
                                     name=f"psA{t_}") for t_ in range(2)]

                    def a_pass(pss, c, m):
                        for t in range(2):
                            nc.tensor.matmul(
                                pss[t][:], xw[:, m, ts(t, 128)],
                                a_sb[:, m, ts(c, 512)],
                                start=(m == 0), stop=(m == 7))

                    xw_pair(0)
                    for c in x0n_layer[layer]:
                        x0n_group(c)
                    xw_pair(1)
                    a_pass(pssA, 0, 0)
                    a_pass(pssA, 0, 1)
                    xw_pair(2)
                    a_pass(pssA, 0, 2)
                    a_pass(pssA, 0, 3)
                    xw_pair(3)
                    for m in range(4, 8):
                        a_pass(pssA, 0, m)
                    for t in range(2):
                        nc.scalar.activation(
                            xn[:, t, ts(0, 512)], pssA[t][:], Relu,
                            bias=b_gcn_pp_sb[:, layer * 2 + t:
                                             layer * 2 + t + 1])

                    pssB = [psx.tile([128, 512], f32, tag="psx",
                                     name=f"psB{t_}") for t_ in range(2)]
                    for m in range(8):
                        a_pass(pssB, 1, m)
                    for t in range(2):
                        nc.scalar.activation(
                            xn[:, t, ts(1, 512)], pssB[t][:], Relu,
                            bias=b_gcn_pp_sb[:, layer * 2 + t:
                                             layer * 2 + t + 1])
                    x = xn

                # final layer: node-major, only the K masked tiles.
                # XW3 m-groups pipeline with the first c-group's A matmuls.
                xw3 = xwpool.tile([128, 8, H], f32r, tag="xw", name="xw3")
                xf = xfpool.tile([128, Kmax, H], f32r, tag="xf", name="xf")

                def xw3_pair(p):
                    ps = psw.tile([128, 2, H], f32, tag="psw",
                                  name="psw3", bufs=2)
                    k = 0
                    for i in range(2):
                        for t in range(2):
                            for hl in range(2):
                                nc.tensor.matmul(
                                    ps[:, i, :],
                                    x[:, t, ts(2 * p + i, 128)],
                                    w_sb[:, (L - 1) * 2 + t, hl, :],
                                    start=(k == 0), stop=(k == 7))
                                k += 1
                    nc.vector.tensor_copy(xw3[:, 2 * p:2 * p + 2, :], ps[:])

                ps3s = {}

                def l3_mm(c, m):
                    if m == 0:
                        ps3s[c] = psw.tile([128, H], f32, tag="ps3",
                                           name="ps3", bufs=2)
                    nc.tensor.matmul(ps3s[c][:], a_sb[:, m, ts(c, 128)],
                                     xw3[:, m, :],
                                     start=(m == 0), stop=False)
                    if m == 7:
                        nc.tensor.matmul(ps3s[c][:], ones_sb[:],
                                         b_g3_row_sb[:],
                                         start=False, stop=True)
                        nc.scalar.activation(xf[:, c, :], ps3s[c][:], Tanh)
                        nc.vector.tensor_tensor(xf[:, c, :], xf[:, c, :],
                                                x0n[:, c, :], add)
                        pmc = psw.tile([128, H], f32, tag="ps3",
                                       name="pmc", bufs=2)
                        nc.tensor.matmul(pmc[0:B, :], m_t_sb[:, off + c, :],
                                         xf[:, c, :], start=True, stop=True)
                        if g == 0 and c == 0:
                            nc.vector.tensor_copy(out_acc[:], pmc[0:B, :])
                        else:
                            nc.vector.tensor_add(out_acc[:], out_acc[:],
                                                 pmc[0:B, :])

                for p in range(4):
                    xw3_pair(p)
                for c in range(K):
                    for m in range(8):
                        l3_mm(c, m)
            # mask columns carry 1/count, so out_acc is the masked mean
            nc.sync.dma_start(out[:, :], out_acc[:])

    nc.compile()
    return nc


def _get_nc(NG, Ks):
    key = (NG, tuple(Ks))
    if key not in _CACHE:
        _CACHE[key] = _build_nc(NG, Ks)
    return _CACHE[key]


def _prepare_in_maps(cdfg_xs, cdfg_as, graph, coverpoint_mask,
                     W_in, b_in, W_gcn, b_gcn):
    cdfg_xs = np.asarray(cdfg_xs, dtype=np.float32)
    cdfg_as = np.asarray(cdfg_as, dtype=np.float32)
    graph = np.asarray(graph).astype(np.int64)
    maskf = np.asarray(coverpoint_mask).astype(np.float32)
    W_in = np.asarray(W_in, dtype=np.float32)
    b_in = np.asarray(b_in, dtype=np.float32)
    W_gcn = np.asarray(W_gcn, dtype=np.float32)
    b_gcn = np.asarray(b_gcn, dtype=np.float32)

    uniq = np.unique(graph)
    u = len(uniq)
    NG = max(1, (u + NCORES - 1) // NCORES)

    # per-graph node permutation (union-masked nodes first) and tile count
    perms, kts = {}, {}
    for gid in uniq:
        um = maskf[graph == gid].any(axis=0)
        perms[int(gid)] = np.argsort(~um, kind="stable")
        kts[int(gid)] = max(1, int(np.ceil(um.sum() / 128)))

    # sort graphs by K desc; rank r -> (slot r//8, core r%8)
    order = sorted(uniq.tolist(), key=lambda g: -kts[int(g)])
    Ks = []
    for s in range(NG):
        bucket = [kts[int(order[r])] for r in range(s * 8, min((s + 1) * 8, u))]
        Ks.append(max(bucket) if bucket else 1)
    T = sum(Ks)
    offs = np.concatenate([[0], np.cumsum(Ks)]).astype(int)

    w_gcn_layout = np.ascontiguousarray(
        W_gcn.reshape(L, 2, 128, H).transpose(2, 0, 1, 3)
        .reshape(128, L * 2, H))
    w_gcn_hi = _rnd11(w_gcn_layout)
    w_gcn_lo = _rnd11(w_gcn_layout - w_gcn_hi)
    w_gcn_pack = np.ascontiguousarray(
        np.stack([w_gcn_hi, w_gcn_lo], axis=2))
    b_pp = np.concatenate([
        b_in.reshape(2, 128).T,
        b_gcn.reshape(L, 2, 128).transpose(2, 0, 1).reshape(128, L * 2)],
        axis=1)
    rows_c = np.concatenate([
        b_in.reshape(1, H), b_gcn[L - 1].reshape(1, H),
        np.ones((1, 128), dtype=np.float32)], axis=1)

    common = {
        "w_in_hi": np.ascontiguousarray(_rnd11(W_in)),
        "w_gcn": w_gcn_pack,
        "b_pp": np.ascontiguousarray(b_pp.astype(np.float32)),
        "rows_c": np.ascontiguousarray(rows_c.astype(np.float32)),
    }

    # per-graph prepped tensors (cached; dead slots reuse order[0])
    a_cache, xs_cache = {}, {}

    def graph_data(gid):
        if gid not in a_cache:
            p = perms[gid]
            a_cache[gid] = np.ascontiguousarray(cdfg_as[gid][p][:, p].T)
            xs_cache[gid] = np.ascontiguousarray(cdfg_xs[gid][p].T)
        return a_cache[gid], xs_cache[gid]

    in_maps = []
    for k in range(NCORES):
        a_t = np.empty((NG, N, N), dtype=np.float32)
        xs_t = np.empty((F, NG, N), dtype=np.float32)
        m_t = np.zeros((128, T, B), dtype=np.float32)
        for s in range(NG):
            r = s * 8 + k
            gid = int(order[r]) if r < u else int(order[0])
            a_g, xs_g = graph_data(gid)
            a_t[s] = a_g
            xs_t[:, s, :] = xs_g
            if r < u:
                p = perms[gid]
                rows = np.nonzero(graph == gid)[0]
                for b in rows:
                    mp = maskf[b][p] / maskf[b].sum()
                    for c in range(kts[gid]):
                        m_t[:, offs[s] + c, b] = mp[c * 128:(c + 1) * 128]
        in_maps.append({"a_t": a_t, "xs_t": xs_t, "m_t": m_t, **common})
    meta = {"NG": NG, "Ks": Ks, "order": order, "u": u}
    return in_maps, meta


def _assemble_out(results, graph, meta):
    graph = np.asarray(graph).astype(np.int64)
    out = np.zeros((B, H), dtype=np.float32)
    for r in range(meta["u"]):
        s, k = r // 8, r % 8
        rows = graph == meta["order"][r]
        out[rows] = results[k]["out"][rows]
    return out


def kernel(cdfg_xs, cdfg_as, graph, coverpoint_mask, W_in, b_in, W_gcn, b_gcn):
    from concourse.bass_utils import run_bass_kernel_spmd

    in_maps, meta = _prepare_in_maps(
        cdfg_xs, cdfg_as, graph, coverpoint_mask, W_in, b_in, W_gcn, b_gcn)
    nc = _get_nc(meta["NG"], meta["Ks"])
    res = run_bass_kernel_spmd(nc, in_maps, core_ids=list(range(NCORES)))
    return _assemble_out(res.results, graph, meta)


# revision 16
# speedup vs baseline: 1.0043x; 1.0043x over previous
"""Trainium2 Bass kernel for CdfgReader GNN message passing.

Strategy:
  - 64 batch items draw from <=32 unique CDFGs: compute the GNN once per
    unique graph; distribute ceil(u/8) graph slots per core across 8 cores
    (SPMD, one compiled program specialized to the input's structure).
  - Error budget (tolerance 2e-2): the end-to-end error is dominated by the
    f32r rounding of the *weights* (a systematic perturbation); activation
    rounding averages out through the A-multiply and the masked mean.
    So W_in/W_gcn ship as f32r hi+lo pairs (every X@W does 2 matmuls per
    contraction tile), while activations stay single f32r and every A-mult
    runs once.  Measured end-to-end ~1.3e-3.
  - Per slot: X0^T = relu(W^T xs^T) h-major; 3x { XW node-major pipelined
    m-outer with the h-major A-multiply X^T = relu(XW^T A^T) }; the final
    layer is computed node-major only for the first K_g 128-node tiles,
    where the host permutes each graph's nodes so the union of its
    coverpoint masks comes first.  The residual relu(xs@W_in+b) is
    recomputed node-major for those K_g tiles directly from xs^T (cheap,
    and it fills the PE while the first slot's A matrix streams in), and
    the masked sums use a small mask matmul.  DMAs are ordered so the
    input-layer operands land first; A streams per 128-row chunk.
"""

import numpy as np

NCORES = 8
N = 1024        # max nodes
F = 128         # input feature dim
H = 256         # hidden dim
L = 4           # GCN layers
B = 64          # batch (coverpoints)

_CACHE = {}


def _rnd11(x):
    # round-to-nearest-even at 11 explicit mantissa bits (f32r-exact)
    m, e = np.frexp(np.float32(x))
    m = np.round(m * 4096.0) / 4096.0
    return np.ldexp(m, e).astype(np.float32)


def _build_nc(NG, Ks):
    import concourse.bass as bass  # noqa: F401
    import concourse.mybir as mybir
    import concourse.tile as tile
    from concourse import bacc
    from concourse.bass import ts

    f32 = mybir.dt.float32
    f32r = mybir.dt.float32r
    Relu = mybir.ActivationFunctionType.Relu
    Tanh = mybir.ActivationFunctionType.Tanh
    add = mybir.AluOpType.add

    T = sum(Ks)
    offs = np.concatenate([[0], np.cumsum(Ks)]).astype(int)
    Kmax = max(Ks)

    nc = bacc.Bacc("TRN2", target_bir_lowering=False, debug=False,
                   num_devices=NCORES)

    a_t = nc.dram_tensor("a_t", [NG, N, N], f32r, kind="ExternalInput")
    xs_t = nc.dram_tensor("xs_t", [F, NG, N], f32r, kind="ExternalInput")
    m_t = nc.dram_tensor("m_t", [128, T, B], f32r, kind="ExternalInput")
    w_in_hi = nc.dram_tensor("w_in_hi", [F, H], f32r, kind="ExternalInput")
    # W_gcn packed [128, (l,t), hi/lo, H] so layer slices are single DMAs
    w_gcn = nc.dram_tensor("w_gcn", [128, L * 2, 2, H], f32r,
                           kind="ExternalInput")
    b_pp = nc.dram_tensor("b_pp", [128, 2 + L * 2], f32, kind="ExternalInput")
    rows_c = nc.dram_tensor("rows_c", [1, 2 * H + 128], f32r,
                            kind="ExternalInput")
    out = nc.dram_tensor("out", [B, H], f32, kind="ExternalOutput")

    with tile.TileContext(nc) as tc:
        with (
            tc.tile_pool(name="const", bufs=1) as constp,
            tc.tile_pool(name="apool", bufs=2) as apool,
            tc.tile_pool(name="xspool", bufs=2) as xspool,
            tc.tile_pool(name="x0pool", bufs=2) as x0pool,
            tc.tile_pool(name="x0npool", bufs=2) as x0npool,
            tc.tile_pool(name="xpool", bufs=2) as xpool,
            tc.tile_pool(name="xwpool", bufs=2) as xwpool,
            tc.tile_pool(name="xfpool", bufs=2) as xfpool,
            tc.tile_pool(name="psx", bufs=4, space="PSUM") as psx,
            tc.tile_pool(name="psw", bufs=3, space="PSUM") as psw,
        ):
            # --- DMA priority order: input-layer operands first, then the
            # first slot's A (chunked), then the rest.
            xs0 = xspool.tile([128, N], f32r, tag="xs", name="xs_g")
            nc.sync.dma_start(xs0[:, 0:512], xs_t[:, 0, 0:512])
            wi_hi_sb = constp.tile([128, H], f32r)
            nc.sync.dma_start(wi_hi_sb[:], w_in_hi[:, :])
            nc.sync.dma_start(xs0[:, 512:1024], xs_t[:, 0, 512:1024])
            b_pp_sb = constp.tile([128, 2 + L * 2], f32)
            nc.sync.dma_start(b_pp_sb[:], b_pp[:, :])
            rows_sb = constp.tile([1, 2 * H + 128], f32r)
            nc.sync.dma_start(rows_sb[:], rows_c[:, :])
            b_in_pp_sb = b_pp_sb[:, 0:2]
            b_gcn_pp_sb = b_pp_sb[:, 2:]
            b_in_row_sb = rows_sb[:, 0:H]
            b_g3_row_sb = rows_sb[:, H:2 * H]
            ones_sb = rows_sb[:, 2 * H:]

            w_sb = constp.tile([128, L * 2, 2, H], f32r)
            # layer-0 slices first (XW0 needs them before a_t finishes)
            nc.sync.dma_start(w_sb[:, 0:2, :, :], w_gcn[:, 0:2, :, :])

            a_sb0 = apool.tile([128, 8, N], f32r, tag="a", name="a_sb")
            for m in range(8):
                nc.sync.dma_start(a_sb0[:, m, :], a_t[0, ts(m, 128), :])

            nc.sync.dma_start(w_sb[:, 2:8, :, :], w_gcn[:, 2:8, :, :])
            m_t_sb = constp.tile([128, T, B], f32r)
            nc.sync.dma_start(m_t_sb[:], m_t[:, :, :])

            out_acc = constp.tile([B, H], f32)

            for g in range(NG):
                K = Ks[g]
                off = int(offs[g])
                if g == 0:
                    a_sb, xs_g = a_sb0, xs0
                else:
                    xs_g = xspool.tile([128, N], f32r, tag="xs", name="xs_g")
                    nc.sync.dma_start(xs_g[:], xs_t[:, g, :])
                    a_sb = apool.tile([128, 8, N], f32r, tag="a", name="a_sb")
                    for m in range(8):
                        nc.sync.dma_start(a_sb[:, m, :], a_t[g, ts(m, 128), :])

                # X0^T h-major [256h x 1024n], relu + bias on ACT
                x0t = x0pool.tile([128, 2, N], f32r, tag="x0")
                for t, c in [(0, 0), (1, 0), (0, 1), (1, 1)]:
                    ps = psx.tile([128, 512], f32, tag="psx", name="ps0")
                    nc.tensor.matmul(ps[:], wi_hi_sb[:, ts(t, 128)],
                                     xs_g[:, ts(c, 512)],
                                     start=True, stop=True)
                    nc.scalar.activation(x0t[:, t, ts(c, 512)], ps[:],
                                         Relu, bias=b_in_pp_sb[:, t:t + 1])

                # residual X0 node-major for the K masked tiles, straight
                # from xs^T; emitted in pieces as PE filler (all upfront for
                # slot 0 -- it hides under the initial A DMA -- else spread
                # across layer boundaries to cover the ACT handoff)
                x0n = x0npool.tile([128, Kmax, H], f32r, tag="x0n", name="x0n")

                def x0n_group(c):
                    ps = psw.tile([128, H], f32, tag="ps3", name="ps0n",
                                  bufs=2)
                    nc.tensor.matmul(ps[:], xs_g[:, ts(c, 128)], wi_hi_sb[:],
                                     start=True, stop=False)
                    nc.tensor.matmul(ps[:], ones_sb[:], b_in_row_sb[:],
                                     start=False, stop=True)
                    nc.scalar.activation(x0n[:, c, :], ps[:], Relu)

                cs = list(range(K))
                if g == 0:
                    # slot 0 is DMA-bound through layer 0: bulk up front,
                    # keep one group for each later layer boundary
                    x0n_layer = {0: [], 1: cs[K - 2:K - 1], 2: cs[K - 1:]}
                    head = cs[:K - 2]
                else:
                    nl = min(3, K - 1)
                    x0n_layer = {l: (cs[K - nl + l:K - nl + l + 1]
                                     if l < nl else [])
                                 for l in range(3)}
                    head = cs[:K - nl]
                for c in head:
                    x0n_group(c)

                x = x0t
                for layer in range(L - 1):
                    # XW node-major (W as hi+lo f32r pair).  The h-major
                    # A-multiply runs as two half-passes: pass A (c0 chunk)
                    # pipelines m-outer with the XW groups, pass B (c1)
                    # streams afterwards while the c0 ACTs drain, so the
                    # next layer's XW never waits on an ACT.
                    xw = xwpool.tile([128, 8, H], f32r, tag="xw", name="xw")
                    xn = xpool.tile([128, 2, N], f32r, tag="xn", name="xn")

                    def xw_pair(p):
                        # two m-tiles share one PSUM bank: one start/stop
                        # group, one wide copy -- halves ring turnarounds
                        ps = psw.tile([128, 2, H], f32, tag="psw",
                                      name="psw", bufs=2)
                        k = 0
                        for i in range(2):
                            for t in range(2):
                                for hl in range(2):
                                    nc.tensor.matmul(
                                        ps[:, i, :],
                                        x[:, t, ts(2 * p + i, 128)],
                                        w_sb[:, layer * 2 + t, hl, :],
                                        start=(k == 0), stop=(k == 7))
                                    k += 1
                        nc.vector.tensor_copy(xw[:, 2 * p:2 * p + 2, :],
                                              ps[:])

                    pssA = [psx.tile([128, 512], f32, tag="psx",
                                     name=f"psA{t_}") for t_ in range(2)]

                    def a_pass(pss, c, m):
                        for t in range(2):
                            nc.tensor.matmul(
                                pss[t][:], xw[:, m, ts(t, 128)],
                                a_sb[:, m, ts(c, 512)],
                                start=(m == 0), stop=(m == 7))

                    xw_pair(0)
                    for c in x0n_layer[layer]:
                        x0n_group(c)
                    xw_pair(1)
                    a_pass(pssA, 0, 0)
                    a_pass(pssA, 0, 1)
                    xw_pair(2)
                    a_pass(pssA, 0, 2)
                    a_pass(pssA, 0, 3)
                    xw_pair(3)
                    for m in range(4, 8):
                        a_pass(pssA, 0, m)
                    for t in range(2):
                        nc.scalar.activation(
                            xn[:, t, ts(0, 512)], pssA[t][:], Relu,
                            bias=b_gcn_pp_sb[:, layer * 2 + t:
                                             layer * 2 + t + 1])

                    pssB = [psx.tile([128, 512], f32, tag="psx",
                                     name=f"psB{t_}") for t_ in range(2)]
                    for m in range(8):
                        a_pass(pssB, 1, m)
                    for t in range(2):
                        nc.scalar.activation(
                            xn[:, t, ts(1, 512)], pssB[t][:], Relu,
                            bias=b_gcn_pp_sb[:, layer * 2 + t:
                                             layer * 2 + t + 1])
                    x = xn

                # final layer: node-major, only the K masked tiles.
                # XW3 m-groups pipeline with the first c-group's A matmuls.
                xw3 = xwpool.tile([128, 8, H], f32r, tag="xw", name="xw3")
                xf = xfpool.tile([128, Kmax, H], f32r, tag="xf", name="xf")

                def xw3_pair(p):
                    ps = psw.tile([128, 2, H], f32, tag="psw",
                                  name="psw3", bufs=2)
                    k = 0
                    for i in range(2):
                        for t in range(2):
                            for hl in range(2):
                                nc.tensor.matmul(
                                    ps[:, i, :],
                                    x[:, t, ts(2 * p + i, 128)],
                                    w_sb[:, (L - 1) * 2 + t, hl, :],
                                    start=(k == 0), stop=(k == 7))
                                k += 1
                    nc.vector.tensor_copy(xw3[:, 2 * p:2 * p + 2, :], ps[:])

                ps3s = {}

                def l3_mm(c, m):
                    if m == 0:
                        ps3s[c] = psw.tile([128, H], f32, tag="ps3",
                                           name="ps3", bufs=2)
                    nc.tensor.matmul(ps3s[c][:], a_sb[:, m, ts(c, 128)],
                                     xw3[:, m, :],
                                     start=(m == 0), stop=False)
                    if m == 7:
                        nc.tensor.matmul(ps3s[c][:], ones_sb[:],
                                         b_g3_row_sb[:],
                                         start=False, stop=True)
                        nc.scalar.activation(xf[:, c, :], ps3s[c][:], Tanh)
                        nc.vector.tensor_tensor(xf[:, c, :], xf[:, c, :],
                                                x0n[:, c, :], add)
                        pmc = psw.tile([128, 2, H], f32, tag="psw",
                                       name="pmc", bufs=2)
                        nc.tensor.matmul(pmc[0:B, 0, :],
                                         m_t_sb[:, off + c, :],
                                         xf[:, c, :], start=True, stop=True)
                        if g == 0 and c == 0:
                            nc.vector.tensor_copy(out_acc[:], pmc[0:B, 0, :])
                        else:
                            nc.vector.tensor_add(out_acc[:], out_acc[:],
                                                 pmc[0:B, 0, :])

                for p in range(4):
                    xw3_pair(p)
                for c in range(K):
                    for m in range(8):
                        l3_mm(c, m)
            # mask columns carry 1/count, so out_acc is the masked mean
            nc.sync.dma_start(out[:, :], out_acc[:])

    nc.compile()
    return nc


def _get_nc(NG, Ks):
    key = (NG, tuple(Ks))
    if key not in _CACHE:
        _CACHE[key] = _build_nc(NG, Ks)
    return _CACHE[key]


def _prepare_in_maps(cdfg_xs, cdfg_as, graph, coverpoint_mask,
                     W_in, b_in, W_gcn, b_gcn):
    cdfg_xs = np.asarray(cdfg_xs, dtype=np.float32)
    cdfg_as = np.asarray(cdfg_as, dtype=np.float32)
    graph = np.asarray(graph).astype(np.int64)
    maskf = np.asarray(coverpoint_mask).astype(np.float32)
    W_in = np.asarray(W_in, dtype=np.float32)
    b_in = np.asarray(b_in, dtype=np.float32)
    W_gcn = np.asarray(W_gcn, dtype=np.float32)
    b_gcn = np.asarray(b_gcn, dtype=np.float32)

    uniq = np.unique(graph)
    u = len(uniq)
    NG = max(1, (u + NCORES - 1) // NCORES)

    # per-graph node permutation (union-masked nodes first) and tile count
    perms, kts = {}, {}
    for gid in uniq:
        um = maskf[graph == gid].any(axis=0)
        perms[int(gid)] = np.argsort(~um, kind="stable")
        kts[int(gid)] = max(1, int(np.ceil(um.sum() / 128)))

    # sort graphs by K desc; rank r -> (slot r//8, core r%8)
    order = sorted(uniq.tolist(), key=lambda g: -kts[int(g)])
    Ks = []
    for s in range(NG):
        bucket = [kts[int(order[r])] for r in range(s * 8, min((s + 1) * 8, u))]
        Ks.append(max(bucket) if bucket else 1)
    T = sum(Ks)
    offs = np.concatenate([[0], np.cumsum(Ks)]).astype(int)

    w_gcn_layout = np.ascontiguousarray(
        W_gcn.reshape(L, 2, 128, H).transpose(2, 0, 1, 3)
        .reshape(128, L * 2, H))
    w_gcn_hi = _rnd11(w_gcn_layout)
    w_gcn_lo = _rnd11(w_gcn_layout - w_gcn_hi)
    w_gcn_pack = np.ascontiguousarray(
        np.stack([w_gcn_hi, w_gcn_lo], axis=2))
    b_pp = np.concatenate([
        b_in.reshape(2, 128).T,
        b_gcn.reshape(L, 2, 128).transpose(2, 0, 1).reshape(128, L * 2)],
        axis=1)
    rows_c = np.concatenate([
        b_in.reshape(1, H), b_gcn[L - 1].reshape(1, H),
        np.ones((1, 128), dtype=np.float32)], axis=1)

    common = {
        "w_in_hi": np.ascontiguousarray(_rnd11(W_in)),
        "w_gcn": w_gcn_pack,
        "b_pp": np.ascontiguousarray(b_pp.astype(np.float32)),
        "rows_c": np.ascontiguousarray(rows_c.astype(np.float32)),
    }

    # per-graph prepped tensors (cached; dead slots reuse order[0])
    a_cache, xs_cache = {}, {}

    def graph_data(gid):
        if gid not in a_cache:
            p = perms[gid]
            a_cache[gid] = np.ascontiguousarray(cdfg_as[gid][p][:, p].T)
            xs_cache[gid] = np.ascontiguousarray(cdfg_xs[gid][p].T)
        return a_cache[gid], xs_cache[gid]

    in_maps = []
    for k in range(NCORES):
        a_t = np.empty((NG, N, N), dtype=np.float32)
        xs_t = np.empty((F, NG, N), dtype=np.float32)
        m_t = np.zeros((128, T, B), dtype=np.float32)
        for s in range(NG):
            r = s * 8 + k
            gid = int(order[r]) if r < u else int(order[0])
            a_g, xs_g = graph_data(gid)
            a_t[s] = a_g
            xs_t[:, s, :] = xs_g
            if r < u:
                p = perms[gid]
                rows = np.nonzero(graph == gid)[0]
                for b in rows:
                    mp = maskf[b][p] / maskf[b].sum()
                    for c in range(kts[gid]):
                        m_t[:, offs[s] + c, b] = mp[c * 128:(c + 1) * 128]
        in_maps.append({"a_t": a_t, "xs_t": xs_t, "m_t": m_t, **common})
    meta = {"NG": NG, "Ks": Ks, "order": order, "u": u}
    return in_maps, meta


def _assemble_out(results, graph, meta):
    graph = np.asarray(graph).astype(np.int64)
    out = np.zeros((B, H), dtype=np.float32)
    for r in range(meta["u"]):
        s, k = r // 8, r % 8
        rows = graph == meta["order"][r]
        out[rows] = results[k]["out"][rows]
    return out


def kernel(cdfg_xs, cdfg_as, graph, coverpoint_mask, W_in, b_in, W_gcn, b_gcn):
    from concourse.bass_utils import run_bass_kernel_spmd

    in_maps, meta = _prepare_in_maps(
        cdfg_xs, cdfg_as, graph, coverpoint_mask, W_in, b_in, W_gcn, b_gcn)
    nc = _get_nc(meta["NG"], meta["Ks"])
    res = run_bass_kernel_spmd(nc, in_maps, core_ids=list(range(NCORES)))
    return _assemble_out(res.results, graph, meta)


# revision 17
# speedup vs baseline: 1.0336x; 1.0291x over previous
"""Trainium2 Bass kernel for CdfgReader GNN message passing.

Strategy:
  - 64 batch items draw from <=32 unique CDFGs: compute the GNN once per
    unique graph; distribute ceil(u/8) graph slots per core across 8 cores
    (SPMD, one compiled program specialized to the input's structure).
  - Error budget (tolerance 2e-2): the end-to-end error is dominated by the
    f32r rounding of the *weights* (a systematic perturbation); activation
    rounding averages out through the A-multiply and the masked mean.
    So W_in/W_gcn ship as f32r hi+lo pairs (every X@W does 2 matmuls per
    contraction tile), while activations stay single f32r and every A-mult
    runs once.  Measured end-to-end ~1.3e-3.
  - Per slot: X0^T = relu(W^T xs^T) h-major; 3x { XW node-major pipelined
    m-outer with the h-major A-multiply X^T = relu(XW^T A^T) }; the final
    layer is computed node-major only for the first K_g 128-node tiles,
    where the host permutes each graph's nodes so the union of its
    coverpoint masks comes first.  The residual relu(xs@W_in+b) is
    recomputed node-major for those K_g tiles directly from xs^T (cheap,
    and it fills the PE while the first slot's A matrix streams in), and
    the masked sums use a small mask matmul.  DMAs are ordered so the
    input-layer operands land first; A streams per 128-row chunk.
"""

import numpy as np

NCORES = 8
N = 1024        # max nodes
F = 128         # input feature dim
H = 256         # hidden dim
L = 4           # GCN layers
B = 64          # batch (coverpoints)

_CACHE = {}


def _rnd11(x):
    # round-to-nearest-even at 11 explicit mantissa bits (f32r-exact)
    m, e = np.frexp(np.float32(x))
    m = np.round(m * 4096.0) / 4096.0
    return np.ldexp(m, e).astype(np.float32)


def _build_nc(NG, Ks):
    import concourse.bass as bass  # noqa: F401
    import concourse.mybir as mybir
    import concourse.tile as tile
    from concourse import bacc
    from concourse.bass import ts

    f32 = mybir.dt.float32
    f32r = mybir.dt.float32r
    Relu = mybir.ActivationFunctionType.Relu
    Tanh = mybir.ActivationFunctionType.Tanh
    add = mybir.AluOpType.add

    T = sum(Ks)
    offs = np.concatenate([[0], np.cumsum(Ks)]).astype(int)
    Kmax = max(Ks)

    nc = bacc.Bacc("TRN2", target_bir_lowering=False, debug=False,
                   num_devices=NCORES)

    a_t = nc.dram_tensor("a_t", [NG, N, N], f32r, kind="ExternalInput")
    xs_t = nc.dram_tensor("xs_t", [F, NG, N], f32r, kind="ExternalInput")
    m_t = nc.dram_tensor("m_t", [128, T, B], f32r, kind="ExternalInput")
    w_in_hi = nc.dram_tensor("w_in_hi", [F, H], f32r, kind="ExternalInput")
    # W_gcn packed [128, (l,t), hi/lo, H] so layer slices are single DMAs
    w_gcn = nc.dram_tensor("w_gcn", [128, L * 2, 2, H], f32r,
                           kind="ExternalInput")
    b_pp = nc.dram_tensor("b_pp", [128, 2 + L * 2], f32, kind="ExternalInput")
    rows_c = nc.dram_tensor("rows_c", [1, 2 * H + 128], f32r,
                            kind="ExternalInput")
    out = nc.dram_tensor("out", [B, H], f32, kind="ExternalOutput")

    with tile.TileContext(nc) as tc:
        with (
            tc.tile_pool(name="const", bufs=1) as constp,
            tc.tile_pool(name="apool", bufs=2) as apool,
            tc.tile_pool(name="xspool", bufs=2) as xspool,
            tc.tile_pool(name="x0pool", bufs=2) as x0pool,
            tc.tile_pool(name="x0npool", bufs=2) as x0npool,
            tc.tile_pool(name="xpool", bufs=2) as xpool,
            tc.tile_pool(name="xwpool", bufs=2) as xwpool,
            tc.tile_pool(name="xfpool", bufs=2) as xfpool,
            tc.tile_pool(name="psx", bufs=4, space="PSUM") as psx,
            tc.tile_pool(name="psw", bufs=3, space="PSUM") as psw,
        ):
            # --- DMA priority order: input-layer operands first, then the
            # first slot's A (chunked), then the rest.
            xs0 = xspool.tile([128, N], f32r, tag="xs", name="xs_g")
            nc.sync.dma_start(xs0[:, 0:512], xs_t[:, 0, 0:512])
            wi_hi_sb = constp.tile([128, H], f32r)
            nc.sync.dma_start(wi_hi_sb[:], w_in_hi[:, :])
            nc.sync.dma_start(xs0[:, 512:1024], xs_t[:, 0, 512:1024])
            b_pp_sb = constp.tile([128, 2 + L * 2], f32)
            nc.sync.dma_start(b_pp_sb[:], b_pp[:, :])
            rows_sb = constp.tile([1, 2 * H + 128], f32r)
            nc.sync.dma_start(rows_sb[:], rows_c[:, :])
            b_in_pp_sb = b_pp_sb[:, 0:2]
            b_gcn_pp_sb = b_pp_sb[:, 2:]
            b_in_row_sb = rows_sb[:, 0:H]
            b_g3_row_sb = rows_sb[:, H:2 * H]
            ones_sb = rows_sb[:, 2 * H:]

            w_sb = constp.tile([128, L * 2, 2, H], f32r)
            # layer-0 slices first (XW0 needs them before a_t finishes)
            nc.sync.dma_start(w_sb[:, 0:2, :, :], w_gcn[:, 0:2, :, :])

            nc.sync.dma_start(w_sb[:, 2:8, :, :], w_gcn[:, 2:8, :, :])
            a_sb0 = apool.tile([128, 8, N], f32r, tag="a", name="a_sb")
            for m in range(8):
                nc.sync.dma_start(a_sb0[:, m, :], a_t[0, ts(m, 128), :])

            m_t_sb = constp.tile([128, T, B], f32r)
            nc.sync.dma_start(m_t_sb[:], m_t[:, :, :])

            out_acc = constp.tile([B, H], f32)

            for g in range(NG):
                K = Ks[g]
                off = int(offs[g])
                if g == 0:
                    a_sb, xs_g = a_sb0, xs0
                else:
                    xs_g = xspool.tile([128, N], f32r, tag="xs", name="xs_g")
                    nc.sync.dma_start(xs_g[:], xs_t[:, g, :])
                    a_sb = apool.tile([128, 8, N], f32r, tag="a", name="a_sb")
                    for m in range(8):
                        nc.sync.dma_start(a_sb[:, m, :], a_t[g, ts(m, 128), :])

                # X0^T h-major [256h x 1024n], relu + bias on ACT
                x0t = x0pool.tile([128, 2, N], f32r, tag="x0")
                for t, c in [(0, 0), (1, 0), (0, 1), (1, 1)]:
                    ps = psx.tile([128, 512], f32, tag="psx", name="ps0")
                    nc.tensor.matmul(ps[:], wi_hi_sb[:, ts(t, 128)],
                                     xs_g[:, ts(c, 512)],
                                     start=True, stop=True)
                    nc.scalar.activation(x0t[:, t, ts(c, 512)], ps[:],
                                         Relu, bias=b_in_pp_sb[:, t:t + 1])

                # residual X0 node-major for the K masked tiles, straight
                # from xs^T; emitted in pieces as PE filler (all upfront for
                # slot 0 -- it hides under the initial A DMA -- else spread
                # across layer boundaries to cover the ACT handoff)
                x0n = x0npool.tile([128, Kmax, H], f32r, tag="x0n", name="x0n")

                def x0n_group(c):
                    ps = psw.tile([128, H], f32, tag="ps3", name="ps0n",
                                  bufs=2)
                    nc.tensor.matmul(ps[:], xs_g[:, ts(c, 128)], wi_hi_sb[:],
                                     start=True, stop=False)
                    nc.tensor.matmul(ps[:], ones_sb[:], b_in_row_sb[:],
                                     start=False, stop=True)
                    nc.scalar.activation(x0n[:, c, :], ps[:], Relu)

                cs = list(range(K))
                if g == 0:
                    # slot 0 is DMA-bound through layer 0: bulk up front,
                    # keep one group for each later layer boundary
                    x0n_layer = {0: [], 1: cs[K - 2:K - 1], 2: cs[K - 1:]}
                    head = cs[:K - 2]
                else:
                    nl = min(3, K - 1)
                    x0n_layer = {l: (cs[K - nl + l:K - nl + l + 1]
                                     if l < nl else [])
                                 for l in range(3)}
                    head = cs[:K - nl]
                for c in head:
                    x0n_group(c)

                x = x0t
                for layer in range(L - 1):
                    # XW node-major (W as hi+lo f32r pair).  The h-major
                    # A-multiply runs as two half-passes: pass A (c0 chunk)
                    # pipelines m-outer with the XW groups, pass B (c1)
                    # streams afterwards while the c0 ACTs drain, so the
                    # next layer's XW never waits on an ACT.
                    xw = xwpool.tile([128, 8, H], f32r, tag="xw", name="xw")
                    xn = xpool.tile([128, 2, N], f32r, tag="xn", name="xn")

                    def xw_pair(p):
                        # two m-tiles share one PSUM bank: one start/stop
                        # group, one wide copy -- halves ring turnarounds
                        ps = psw.tile([128, 2, H], f32, tag="psw",
                                      name="psw", bufs=2)
                        k = 0
                        for i in range(2):
                            for t in range(2):
                                for hl in range(2):
                                    nc.tensor.matmul(
                                        ps[:, i, :],
                                        x[:, t, ts(2 * p + i, 128)],
                                        w_sb[:, layer * 2 + t, hl, :],
                                        start=(k == 0), stop=(k == 7))
                                    k += 1
                        nc.vector.tensor_copy(xw[:, 2 * p:2 * p + 2, :],
                                              ps[:])

                    pssA = [psx.tile([128, 512], f32, tag="psx",
                                     name=f"psA{t_}") for t_ in range(2)]

                    def a_pass(pss, c, m):
                        for t in range(2):
                            nc.tensor.matmul(
                                pss[t][:], xw[:, m, ts(t, 128)],
                                a_sb[:, m, ts(c, 512)],
                                start=(m == 0), stop=(m == 7))

                    xw_pair(0)
                    for c in x0n_layer[layer]:
                        x0n_group(c)
                    xw_pair(1)
                    a_pass(pssA, 0, 0)
                    a_pass(pssA, 0, 1)
                    xw_pair(2)
                    a_pass(pssA, 0, 2)
                    a_pass(pssA, 0, 3)
                    xw_pair(3)
                    for m in range(4, 8):
                        a_pass(pssA, 0, m)
                    for t in range(2):
                        nc.scalar.activation(
                            xn[:, t, ts(0, 512)], pssA[t][:], Relu,
                            bias=b_gcn_pp_sb[:, layer * 2 + t:
                                             layer * 2 + t + 1])

                    pssB = [psx.tile([128, 512], f32, tag="psx",
                                     name=f"psB{t_}") for t_ in range(2)]
                    for m in range(8):
                        a_pass(pssB, 1, m)
                    for t in range(2):
                        nc.scalar.activation(
                            xn[:, t, ts(1, 512)], pssB[t][:], Relu,
                            bias=b_gcn_pp_sb[:, layer * 2 + t:
                                             layer * 2 + t + 1])
                    x = xn

                # final layer: node-major, only the K masked tiles.
                # XW3 m-groups pipeline with the first c-group's A matmuls.
                xw3 = xwpool.tile([128, 8, H], f32r, tag="xw", name="xw3")
                xf = xfpool.tile([128, Kmax, H], f32r, tag="xf", name="xf")

                def xw3_pair(p):
                    ps = psw.tile([128, 2, H], f32, tag="psw",
                                  name="psw3", bufs=2)
                    k = 0
                    for i in range(2):
                        for t in range(2):
                            for hl in range(2):
                                nc.tensor.matmul(
                                    ps[:, i, :],
                                    x[:, t, ts(2 * p + i, 128)],
                                    w_sb[:, (L - 1) * 2 + t, hl, :],
                                    start=(k == 0), stop=(k == 7))
                                k += 1
                    nc.vector.tensor_copy(xw3[:, 2 * p:2 * p + 2, :], ps[:])

                ps3s = {}

                def l3_mm(c, m):
                    if m == 0:
                        ps3s[c] = psw.tile([128, H], f32, tag="ps3",
                                           name="ps3", bufs=2)
                    nc.tensor.matmul(ps3s[c][:], a_sb[:, m, ts(c, 128)],
                                     xw3[:, m, :],
                                     start=(m == 0), stop=False)
                    if m == 7:
                        nc.tensor.matmul(ps3s[c][:], ones_sb[:],
                                         b_g3_row_sb[:],
                                         start=False, stop=True)
                        nc.scalar.activation(xf[:, c, :], ps3s[c][:], Tanh)
                        nc.vector.tensor_tensor(xf[:, c, :], xf[:, c, :],
                                                x0n[:, c, :], add)
                        pmc = psw.tile([128, 2, H], f32, tag="psw",
                                       name="pmc", bufs=2)
                        nc.tensor.matmul(pmc[0:B, 0, :],
                                         m_t_sb[:, off + c, :],
                                         xf[:, c, :], start=True, stop=True)
                        if g == 0 and c == 0:
                            nc.vector.tensor_copy(out_acc[:], pmc[0:B, 0, :])
                        else:
                            nc.vector.tensor_add(out_acc[:], out_acc[:],
                                                 pmc[0:B, 0, :])

                for p in range(4):
                    xw3_pair(p)
                for c in range(K):
                    for m in range(8):
                        l3_mm(c, m)
            # mask columns carry 1/count, so out_acc is the masked mean
            nc.sync.dma_start(out[:, :], out_acc[:])

    nc.compile()
    return nc


def _get_nc(NG, Ks):
    key = (NG, tuple(Ks))
    if key not in _CACHE:
        _CACHE[key] = _build_nc(NG, Ks)
    return _CACHE[key]


def _prepare_in_maps(cdfg_xs, cdfg_as, graph, coverpoint_mask,
                     W_in, b_in, W_gcn, b_gcn):
    cdfg_xs = np.asarray(cdfg_xs, dtype=np.float32)
    cdfg_as = np.asarray(cdfg_as, dtype=np.float32)
    graph = np.asarray(graph).astype(np.int64)
    maskf = np.asarray(coverpoint_mask).astype(np.float32)
    W_in = np.asarray(W_in, dtype=np.float32)
    b_in = np.asarray(b_in, dtype=np.float32)
    W_gcn = np.asarray(W_gcn, dtype=np.float32)
    b_gcn = np.asarray(b_gcn, dtype=np.float32)

    uniq = np.unique(graph)
    u = len(uniq)
    NG = max(1, (u + NCORES - 1) // NCORES)

    # per-graph node permutation (union-masked nodes first) and tile count
    perms, kts = {}, {}
    for gid in uniq:
        um = maskf[graph == gid].any(axis=0)
        perms[int(gid)] = np.argsort(~um, kind="stable")
        kts[int(gid)] = max(1, int(np.ceil(um.sum() / 128)))

    # sort graphs by K desc; rank r -> (slot r//8, core r%8)
    order = sorted(uniq.tolist(), key=lambda g: -kts[int(g)])
    Ks = []
    for s in range(NG):
        bucket = [kts[int(order[r])] for r in range(s * 8, min((s + 1) * 8, u))]
        Ks.append(max(bucket) if bucket else 1)
    T = sum(Ks)
    offs = np.concatenate([[0], np.cumsum(Ks)]).astype(int)

    w_gcn_layout = np.ascontiguousarray(
        W_gcn.reshape(L, 2, 128, H).transpose(2, 0, 1, 3)
        .reshape(128, L * 2, H))
    w_gcn_hi = _rnd11(w_gcn_layout)
    w_gcn_lo = _rnd11(w_gcn_layout - w_gcn_hi)
    w_gcn_pack = np.ascontiguousarray(
        np.stack([w_gcn_hi, w_gcn_lo], axis=2))
    b_pp = np.concatenate([
        b_in.reshape(2, 128).T,
        b_gcn.reshape(L, 2, 128).transpose(2, 0, 1).reshape(128, L * 2)],
        axis=1)
    rows_c = np.concatenate([
        b_in.reshape(1, H), b_gcn[L - 1].reshape(1, H),
        np.ones((1, 128), dtype=np.float32)], axis=1)

    common = {
        "w_in_hi": np.ascontiguousarray(_rnd11(W_in)),
        "w_gcn": w_gcn_pack,
        "b_pp": np.ascontiguousarray(b_pp.astype(np.float32)),
        "rows_c": np.ascontiguousarray(rows_c.astype(np.float32)),
    }

    # per-graph prepped tensors (cached; dead slots reuse order[0])
    a_cache, xs_cache = {}, {}

    def graph_data(gid):
        if gid not in a_cache:
            p = perms[gid]
            a_cache[gid] = np.ascontiguousarray(cdfg_as[gid][p][:, p].T)
            xs_cache[gid] = np.ascontiguousarray(cdfg_xs[gid][p].T)
        return a_cache[gid], xs_cache[gid]

    in_maps = []
    for k in range(NCORES):
        a_t = np.empty((NG, N, N), dtype=np.float32)
        xs_t = np.empty((F, NG, N), dtype=np.float32)
        m_t = np.zeros((128, T, B), dtype=np.float32)
        for s in range(NG):
            r = s * 8 + k
            gid = int(order[r]) if r < u else int(order[0])
            a_g, xs_g = graph_data(gid)
            a_t[s] = a_g
            xs_t[:, s, :] = xs_g
            if r < u:
                p = perms[gid]
                rows = np.nonzero(graph == gid)[0]
                for b in rows:
                    mp = maskf[b][p] / maskf[b].sum()
                    for c in range(kts[gid]):
                        m_t[:, offs[s] + c, b] = mp[c * 128:(c + 1) * 128]
        in_maps.append({"a_t": a_t, "xs_t": xs_t, "m_t": m_t, **common})
    meta = {"NG": NG, "Ks": Ks, "order": order, "u": u}
    return in_maps, meta


def _assemble_out(results, graph, meta):
    graph = np.asarray(graph).astype(np.int64)
    out = np.zeros((B, H), dtype=np.float32)
    for r in range(meta["u"]):
        s, k = r // 8, r % 8
        rows = graph == meta["order"][r]
        out[rows] = results[k]["out"][rows]
    return out


def kernel(cdfg_xs, cdfg_as, graph, coverpoint_mask, W_in, b_in, W_gcn, b_gcn):
    from concourse.bass_utils import run_bass_kernel_spmd

    in_maps, meta = _prepare_in_maps(
        cdfg_xs, cdfg_as, graph, coverpoint_mask, W_in, b_in, W_gcn, b_gcn)
    nc = _get_nc(meta["NG"], meta["Ks"])
    res = run_bass_kernel_spmd(nc, in_maps, core_ids=list(range(NCORES)))
    return _assemble_out(res.results, graph, meta)
